# revision 2
# baseline (speedup 1.0000x reference)
"""Single-head causal attention on 8 Trainium2 NeuronCores (Bass/Tile).

Problem: x [4, 4096, 1024] f32, Wq/Wk/Wv [64, 1024] f32 ->
         softmax(causal(q k^T * H^-0.5)) v   -> [4, 4096, 64] f32

Sharding: core = (batch b, parity p), b = core//2, p = core%2. Each core owns
the global 128-wide query tiles g = 2j+p (j=0..15) of its batch -- the
parity interleave balances causal work AND keeps the compiled graph identical
across all 8 cores (SPMD: one NEFF). All parity differences live in
host-prepared input data (xtq ordering, mask contents), never in the graph.

Device pipeline (bf16 matmuls, f32 PSUM accumulation):
  1. x^T / x_q^T loaded fully SBUF-resident via per-(c-tile, t-chunk) DMAs.
  2. Q^T projection with duplicated weights [Wq.T|Wq.T]; fused [V^T;K^T]
     projection (lhsT = [Wv.T|Wk.T]); K^T repartitioned to rows 0:64 via
     SBUF->SBUF DMA; V^T -> V via PE transposes with a ones-column appended
     so the softmax denominator falls out of the AV matmul.
  3. Per 512-wide q-chunk ch (extent 8ch+8 k-tiles, processed in pairs):
     S^T tiles [128k, 512q] on PE -> exp on ScalarE (scale folded) -> causal
     masking (data-driven masks on GpSimd) -> AV accumulation.
     Projection work for phase ch+1 is drip-fed between attention groups so
     the ScalarE exp stream (the critical engine) never stalls.
  4. Epilogue per chunk: PE transpose [65,128]->[128,65], reciprocal of the
     denominator column, scale, DMA out.
"""
import os

import numpy as np
import ml_dtypes

import concourse.bass as bass
import concourse.mybir as mybir
import concourse.tile as tile
from concourse import bacc
from concourse.bass_utils import run_bass_kernel_spmd
from concourse.masks import make_identity

P = 128
B, T, C, H = 4, 4096, 1024, 64
TQ = T // 2          # queries per core
CH = 512             # q-chunk width
NCH = TQ // CH       # 4 q-chunks
CT = C // P          # 8 contraction tiles
TC = T // CH         # 8 t-chunks for K/V proj
NKT = T // P         # 32 k-tiles
GROUP = 2            # k-tiles per exp group
N_CORES = 8

F32 = mybir.dt.float32
BF16 = mybir.dt.bfloat16
Exp = mybir.ActivationFunctionType.Exp
MULT = mybir.AluOpType.mult

LAST_EXEC_TIME_NS = None
_COMPILED = None


def _build_graph():
    nc = bacc.Bacc("TRN2", target_bir_lowering=False, debug=False,
                   num_devices=N_CORES)
    xt = nc.dram_tensor("xt", [C, T], BF16, kind="ExternalInput").ap()
    xtq = nc.dram_tensor("xtq", [C, TQ], BF16, kind="ExternalInput").ap()
    wqq = nc.dram_tensor("wqq", [C, P], BF16, kind="ExternalInput").ap()
    wkv = nc.dram_tensor("wkv", [C, P], BF16, kind="ExternalInput").ap()
    m01 = nc.dram_tensor("m01", [P, 2 * P], BF16, kind="ExternalInput").ap()
    y = nc.dram_tensor("y", [TQ, H], F32, kind="ExternalOutput").ap()

    xt_r = xt.rearrange("(co p) t -> p co t", p=P)     # [128, 8, 4096]
    xtq_r = xtq.rearrange("(co p) t -> p co t", p=P)   # [128, 8, 2048]
    wqq_r = wqq.rearrange("(co p) m -> p co m", p=P)   # [128, 8, 128]
    wkv_r = wkv.rearrange("(co p) m -> p co m", p=P)

    with tile.TileContext(nc) as tc:
        with (
            tc.tile_pool(name="const", bufs=1) as const,
            tc.tile_pool(name="ssb", bufs=3) as sspool,
            tc.tile_pool(name="epi", bufs=2) as epool,
            tc.tile_pool(name="pproj", bufs=1, space="PSUM") as ppool,
            tc.tile_pool(name="ps", bufs=2, space="PSUM") as spool,
            tc.tile_pool(name="po", bufs=2, space="PSUM") as opool,
            tc.tile_pool(name="pt", bufs=1, space="PSUM") as tpool,
        ):
            # ---- constants ----
            wqq_sb = const.tile([P, CT, P], BF16, name="wqq_sb")
            wkv_sb = const.tile([P, CT, P], BF16, name="wkv_sb")
            mask_sb = const.tile([P, 2, P], BF16, name="mask_sb")
            ident16 = const.tile([P, P], BF16, name="ident16")
            ident32 = const.tile([P, P], F32, name="ident32")
            scratch = const.tile([P, 1], F32, name="scratch")
            nc.sync.dma_start(wqq_sb[:], wqq_r)
            nc.sync.dma_start(wkv_sb[:], wkv_r)
            nc.sync.dma_start(mask_sb[:], m01.rearrange("p (two m) -> p two m", two=2))
            make_identity(nc, ident16[:])
            make_identity(nc, ident32[:])
            # preload the exp table set while projections run
            nc.scalar.activation(scratch[:], ident32[:, 0:1], Exp)

            # ---- resident x ----
            xt_sb = const.tile([P, CT, T], BF16, name="xt_sb")
            xtq_sb = const.tile([P, CT, TQ], BF16, name="xtq_sb")

            # ---- persistent activations ----
            qt_sb = const.tile([P, TQ], BF16, name="qt_sb")      # Q^T dup rows
            kt_sb = const.tile([P, T], BF16, name="kt_sb")       # K^T top, zero bottom
            kstage = const.tile([P, T], BF16, name="kstage")     # K^T at rows 64:128
            vt_sb = const.tile([64, T], BF16, name="vt_sb")      # V^T
            v_sb = const.tile([P, NKT, H + 1], BF16, name="v_sb")  # V tiles + ones

            nc.gpsimd.memset(kt_sb[64:128, :], 0.0)
            nc.gpsimd.memset(v_sb[:, :, H:H + 1], 1.0)

            # ---- DMA schedule: everything up-front, consumption-ordered ----
            for qc in range(NCH):
                for c in range(CT):
                    nc.sync.dma_start(xtq_sb[:, c, bass.ts(qc, CH)],
                                      xtq_r[:, c, bass.ts(qc, CH)])
                for t_i in (2 * qc, 2 * qc + 1):
                    for c in range(CT):
                        nc.sync.dma_start(xt_sb[:, c, bass.ts(t_i, CH)],
                                          xt_r[:, c, bass.ts(t_i, CH)])

            # ---- projection work units (drip-fed between attention groups) --
            def q_proj_units(qc):
                ps = ppool.tile([P, CH], F32, tag="ps_proj")
                for c in range(CT):
                    yield lambda c=c, ps=ps: nc.tensor.matmul(
                        ps[:], lhsT=wqq_sb[:, c, :],
                        rhs=xtq_sb[:, c, bass.ts(qc, CH)],
                        start=(c == 0), stop=(c == CT - 1))
                yield lambda ps=ps: nc.vector.tensor_copy(
                    qt_sb[:, bass.ts(qc, CH)], ps[:])

            def kv_proj_units(t_i):
                ps = ppool.tile([P, CH], F32, tag="ps_proj")
                for c in range(CT):
                    yield lambda c=c, ps=ps: nc.tensor.matmul(
                        ps[:], lhsT=wkv_sb[:, c, :],
                        rhs=xt_sb[:, c, bass.ts(t_i, CH)],
                        start=(c == 0), stop=(c == CT - 1))

                def evac(ps=ps):
                    nc.vector.tensor_copy(vt_sb[:, bass.ts(t_i, CH)], ps[0:64, :])
                    nc.vector.tensor_copy(kstage[64:128, bass.ts(t_i, CH)],
                                          ps[64:128, :])
                    nc.sync.dma_start(kt_sb[0:64, bass.ts(t_i, CH)],
                                      kstage[64:128, bass.ts(t_i, CH)])
                yield evac
                for j in range(CH // P):
                    def vtile(j=j):
                        kt = t_i * (CH // P) + j
                        pt = tpool.tile([P, P], BF16, tag="tr")
                        nc.tensor.transpose(pt[:, 0:64], vt_sb[:, bass.ts(kt, P)],
                                            ident16[0:64, 0:64])
                        nc.vector.tensor_copy(v_sb[:, kt, 0:H], pt[:, 0:64])
                    yield vtile

            def phase_units(phase):
                # work that must be complete before attention chunk `phase`
                if phase == 0:
                    yield from q_proj_units(0)
                    yield from kv_proj_units(0)
                    yield from kv_proj_units(1)
                else:
                    yield from q_proj_units(phase)
                    yield from kv_proj_units(2 * phase)
                    yield from kv_proj_units(2 * phase + 1)

            # ---- attention ----
            for u in phase_units(0):
                u()
            feeder = None
            for ch in range(NCH):
                ext = 8 * ch + 8
                n_groups = ext // GROUP
                if ch + 1 < NCH:
                    feeder = phase_units(ch + 1)
                    # spread next phase's ~26 units over this chunk's groups
                    per_group = 26 // n_groups + 1
                else:
                    feeder = None
                po = opool.tile([P, CH], F32, name="po")
                for g in range(n_groups):
                    ps_s = spool.tile([P, GROUP * CH], F32, name="ps_s")
                    s_sb = sspool.tile([P, GROUP * CH], BF16, tag="s_sb")
                    for j in range(GROUP):
                        kt = g * GROUP + j
                        nc.tensor.matmul(
                            ps_s[:, bass.ts(j, CH)],
                            lhsT=kt_sb[:, bass.ts(kt, P)],
                            rhs=qt_sb[:, bass.ts(ch, CH)],
                            start=True, stop=True)
                    nc.scalar.activation(s_sb[:], ps_s[:], Exp, scale=0.125)
                    for j in range(GROUP):
                        kt = g * GROUP + j
                        for r in range(4):
                            d = kt - 8 * ch - 2 * r
                            if d < 0:
                                continue
                            blk = s_sb[:, j * CH + r * P: j * CH + (r + 1) * P]
                            if d <= 1:
                                nc.gpsimd.tensor_tensor(blk, blk, mask_sb[:, d, :], MULT)
                            else:
                                nc.gpsimd.memset(blk, 0.0)
                    for j in range(GROUP):
                        kt = g * GROUP + j
                        nc.tensor.matmul(po[0:H + 1, :],
                                         lhsT=v_sb[:, kt, :],
                                         rhs=s_sb[:, bass.ts(j, CH)],
                                         start=(kt == 0), stop=(kt == ext - 1))
                    if feeder is not None:
                        for _ in range(per_group):
                            u = next(feeder, None)
                            if u is None:
                                break
                            u()
                if feeder is not None:
                    for u in feeder:
                        u()
                # epilogue
                osb = epool.tile([H + 1, CH], F32, tag="osb")
                nc.vector.tensor_copy(osb[:], po[0:H + 1, :])
                for s in range(CH // P):
                    pt2 = tpool.tile([P, P], F32, tag="tr")
                    nc.tensor.transpose(pt2[:, 0:H + 1], osb[:, bass.ts(s, P)],
                                        ident32[0:H + 1, 0:H + 1])
                    rec = epool.tile([P, 1], F32, tag="rec")
                    nc.vector.reciprocal(rec[:], pt2[:, H:H + 1])
                    ot = epool.tile([P, H], F32, tag="ot")
                    nc.vector.tensor_scalar_mul(ot[:], pt2[:, 0:H], rec[:])
                    nc.sync.dma_start(y[bass.ds(ch * CH + s * P, P), :], ot[:])

    nc.compile()
    return nc


def _shard_inputs(x, Wq, Wk, Wv):
    bf = ml_dtypes.bfloat16
    tri = np.tril(np.ones((P, P), dtype=np.float32)).T  # [kk,qq]=1 iff kk<=qq
    ones = np.ones((P, P), dtype=np.float32)
    zeros = np.zeros((P, P), dtype=np.float32)
    wqq = np.concatenate([Wq.T, Wq.T], axis=1).astype(bf)
    wkv = np.concatenate([Wv.T, Wk.T], axis=1).astype(bf)
    in_maps = []
    for core in range(N_CORES):
        b, p = core // 2, core % 2
        xt = np.ascontiguousarray(x[b].T).astype(bf)
        idx = np.concatenate([np.arange(P * (2 * j + p), P * (2 * j + p) + P)
                              for j in range(16)])
        xtq = np.ascontiguousarray(x[b][idx].T).astype(bf)
        m0 = tri if p == 0 else ones
        m1 = zeros if p == 0 else tri
        m01 = np.concatenate([m0, m1], axis=1).astype(bf)
        in_maps.append({"xt": xt, "xtq": xtq, "wqq": wqq, "wkv": wkv, "m01": m01})
    return in_maps


def _unshard(results):
    y = np.zeros((B, T, H), dtype=np.float32)
    for core in range(N_CORES):
        b, p = core // 2, core % 2
        yc = results[core]["y"]
        for j in range(16):
            g = 2 * j + p
            y[b, P * g:P * g + P] = yc[P * j:P * j + P]
    return y


def kernel(x, Wq, Wk, Wv):
    global LAST_EXEC_TIME_NS, _COMPILED
    x = np.asarray(x, dtype=np.float32)
    Wq = np.asarray(Wq, dtype=np.float32)
    Wk = np.asarray(Wk, dtype=np.float32)
    Wv = np.asarray(Wv, dtype=np.float32)

    if _COMPILED is None:
        _COMPILED = _build_graph()
    nc = _COMPILED

    in_maps = _shard_inputs(x, Wq, Wk, Wv)
    kwargs = {}
    if os.environ.get("ATTN_TRACE"):
        kwargs["trace"] = True
        if os.environ.get("ATTN_TRACE_DIR"):
            kwargs["tmpdir"] = os.environ["ATTN_TRACE_DIR"]
    res = run_bass_kernel_spmd(nc, in_maps, core_ids=list(range(N_CORES)), **kwargs)
    LAST_EXEC_TIME_NS = res.exec_time_ns
    return _unshard(res.results)


# revision 5
# speedup vs baseline: 1.2355x; 1.2355x over previous
"""Single-head causal attention on 8 Trainium2 NeuronCores (Bass/Tile).

Problem: x [4, 4096, 1024] f32, Wq/Wk/Wv [64, 1024] f32 ->
         softmax(causal(q k^T * H^-0.5)) v   -> [4, 4096, 64] f32

Sharding: core = (batch b, parity p), b = core//2, p = core%2. Each core owns
the global 128-wide query tiles g = 2j+p (j=0..15) of its batch -- the
parity interleave balances causal work AND keeps the compiled graph identical
across all 8 cores (SPMD: one NEFF). All parity differences live in
host-prepared input data (xtq ordering, mask contents), never in the graph.

Device pipeline (bf16 matmuls, f32 PSUM accumulation):
  1. x^T / x_q^T loaded fully SBUF-resident via per-(c-tile, t-chunk) DMAs.
  2. Q^T projection with duplicated weights [Wq.T|Wq.T]; fused [V^T;K^T]
     projection (lhsT = [Wv.T|Wk.T]); K^T repartitioned to rows 0:64 via
     SBUF->SBUF DMA; V^T -> V via PE transposes with a ones-column appended
     so the softmax denominator falls out of the AV matmul.
  3. Per 512-wide q-chunk ch (extent 8ch+8 k-tiles, processed in pairs):
     S^T tiles [128k, 512q] on PE -> exp on ScalarE (scale folded) -> causal
     masking (data-driven masks on GpSimd) -> AV accumulation.
     Projection work for phase ch+1 is drip-fed between attention groups so
     the ScalarE exp stream (the critical engine) never stalls.
  4. Epilogue per chunk: PE transpose [65,128]->[128,65], reciprocal of the
     denominator column, scale, DMA out.
"""
import os

import numpy as np
import ml_dtypes

import concourse.bass as bass
import concourse.mybir as mybir
import concourse.tile as tile
from concourse import bacc
from concourse.bass_utils import run_bass_kernel_spmd
from concourse.masks import make_identity

P = 128
B, T, C, H = 4, 4096, 1024, 64
TQ = T // 2          # queries per core
CH = 512             # q-chunk width
NCH = TQ // CH       # 4 q-chunks
CT = C // P          # 8 contraction tiles
TC = T // CH         # 8 t-chunks for K/V proj
NKT = T // P         # 32 k-tiles
GROUP = 2            # k-tiles per exp group
N_CORES = 8

F32 = mybir.dt.float32
BF16 = mybir.dt.bfloat16
Exp = mybir.ActivationFunctionType.Exp
MULT = mybir.AluOpType.mult

LAST_EXEC_TIME_NS = None
_COMPILED = None


def _build_graph():
    nc = bacc.Bacc("TRN2", target_bir_lowering=False, debug=False,
                   num_devices=N_CORES)
    xt = nc.dram_tensor("xt", [C, T], BF16, kind="ExternalInput").ap()
    xtq = nc.dram_tensor("xtq", [C, TQ], BF16, kind="ExternalInput").ap()
    wqq = nc.dram_tensor("wqq", [C, P], BF16, kind="ExternalInput").ap()
    wkv = nc.dram_tensor("wkv", [C, P], BF16, kind="ExternalInput").ap()
    m01 = nc.dram_tensor("m01", [P, 2 * P], BF16, kind="ExternalInput").ap()
    y = nc.dram_tensor("y", [TQ, H], F32, kind="ExternalOutput").ap()

    xt_r = xt.rearrange("(co p) t -> p co t", p=P)     # [128, 8, 4096]
    xtq_r = xtq.rearrange("(co p) t -> p co t", p=P)   # [128, 8, 2048]
    wqq_r = wqq.rearrange("(co p) m -> p co m", p=P)   # [128, 8, 128]
    wkv_r = wkv.rearrange("(co p) m -> p co m", p=P)

    with tile.TileContext(nc) as tc:
        with (
            tc.tile_pool(name="const", bufs=1) as const,
            tc.tile_pool(name="ssb", bufs=3) as sspool,
            tc.tile_pool(name="epi", bufs=2) as epool,
            tc.tile_pool(name="pproj", bufs=1, space="PSUM") as ppool,
            tc.tile_pool(name="ps", bufs=2, space="PSUM") as spool,
            tc.tile_pool(name="po", bufs=2, space="PSUM") as opool,
            tc.tile_pool(name="pt", bufs=1, space="PSUM") as tpool,
        ):
            # ---- constants ----
            wqq_sb = const.tile([P, CT, P], BF16, name="wqq_sb")
            wkv_sb = const.tile([P, CT, P], BF16, name="wkv_sb")
            mask_sb = const.tile([P, 2, P], BF16, name="mask_sb")
            ident16 = const.tile([P, P], BF16, name="ident16")
            ident32 = const.tile([P, P], F32, name="ident32")
            scratch = const.tile([P, 1], F32, name="scratch")
            nc.sync.dma_start(wqq_sb[:], wqq_r)
            nc.sync.dma_start(wkv_sb[:], wkv_r)
            nc.sync.dma_start(mask_sb[:], m01.rearrange("p (two m) -> p two m", two=2))
            make_identity(nc, ident16[:])
            make_identity(nc, ident32[:])
            # preload the exp table set while projections run
            nc.scalar.activation(scratch[:], ident32[:, 0:1], Exp)

            # ---- resident x ----
            xt_sb = const.tile([P, CT, T], BF16, name="xt_sb")
            xtq_sb = const.tile([P, CT, TQ], BF16, name="xtq_sb")

            # ---- persistent activations ----
            qt_sb = const.tile([P, TQ], BF16, name="qt_sb")      # Q^T dup rows
            kt_sb = const.tile([P, T], BF16, name="kt_sb")       # K^T top, zero bottom
            kstage = const.tile([P, T], BF16, name="kstage")     # K^T at rows 64:128
            vt_sb = const.tile([64, T], BF16, name="vt_sb")      # V^T
            v_sb = const.tile([P, NKT, H + 1], BF16, name="v_sb")  # V tiles + ones

            nc.gpsimd.memset(kt_sb[64:128, :], 0.0)
            nc.gpsimd.memset(v_sb[:, :, H:H + 1], 1.0)

            # ---- DMA schedule: big lines, consumption-ordered ----
            # phase-0 needs: xtq cols 0:512 (Q0) and xt cols 0:1024 (KV0/1)
            for c in range(CT):
                nc.sync.dma_start(xtq_sb[:, c, 0:CH], xtq_r[:, c, 0:CH])
            for c in range(CT):
                nc.sync.dma_start(xt_sb[:, c, 0:2 * CH], xt_r[:, c, 0:2 * CH])
            for c in range(CT):
                nc.sync.dma_start(xtq_sb[:, c, CH:TQ], xtq_r[:, c, CH:TQ])
            for c in range(CT):
                nc.sync.dma_start(xt_sb[:, c, 2 * CH:T], xt_r[:, c, 2 * CH:T])

            # ---- projection work units (drip-fed between attention groups) --
            def q_proj_units(qc):
                ps = ppool.tile([P, CH], F32, tag="ps_proj")
                for c in range(CT):
                    yield lambda c=c, ps=ps: nc.tensor.matmul(
                        ps[:], lhsT=wqq_sb[:, c, :],
                        rhs=xtq_sb[:, c, bass.ts(qc, CH)],
                        start=(c == 0), stop=(c == CT - 1))
                yield lambda ps=ps: nc.vector.tensor_copy(
                    qt_sb[:, bass.ts(qc, CH)], ps[:])

            def kv_proj_units(t_i):
                ps = ppool.tile([P, CH], F32, tag="ps_proj")
                for c in range(CT):
                    yield lambda c=c, ps=ps: nc.tensor.matmul(
                        ps[:], lhsT=wkv_sb[:, c, :],
                        rhs=xt_sb[:, c, bass.ts(t_i, CH)],
                        start=(c == 0), stop=(c == CT - 1))

                def evac(ps=ps):
                    nc.vector.tensor_copy(vt_sb[:, bass.ts(t_i, CH)], ps[0:64, :])
                    nc.vector.tensor_copy(kstage[64:128, bass.ts(t_i, CH)],
                                          ps[64:128, :])
                    nc.gpsimd.dma_start(kt_sb[0:64, bass.ts(t_i, CH)],
                                        kstage[64:128, bass.ts(t_i, CH)])
                yield evac
                for j in range(CH // P):
                    def vtile(j=j):
                        kt = t_i * (CH // P) + j
                        pt = tpool.tile([P, P], BF16, tag="tr")
                        nc.tensor.transpose(pt[:, 0:64], vt_sb[:, bass.ts(kt, P)],
                                            ident16[0:64, 0:64])
                        nc.vector.tensor_copy(v_sb[:, kt, 0:H], pt[:, 0:64])
                    yield vtile

            def phase_units(phase):
                # work that must be complete before attention chunk `phase`
                if phase == 0:
                    yield from q_proj_units(0)
                    yield from kv_proj_units(0)
                    yield from kv_proj_units(1)
                else:
                    yield from q_proj_units(phase)
                    yield from kv_proj_units(2 * phase)
                    yield from kv_proj_units(2 * phase + 1)

            # ---- attention ----
            for u in phase_units(0):
                u()
            feeder = None
            for ch in range(NCH):
                ext = 8 * ch + 8
                n_groups = ext // GROUP
                if ch + 1 < NCH:
                    feeder = phase_units(ch + 1)
                    # spread next phase's ~26 units over this chunk's groups
                    per_group = 26 // n_groups + 1
                else:
                    feeder = None
                po = opool.tile([P, CH], F32, name="po")
                for g in range(n_groups):
                    # left col-blocks with d = kt-8ch-2r >= 2 are fully masked:
                    # skip them in S^T, exp, and AV (suffix slicing). Both
                    # k-tiles of the pair share r0 = max(0, g - 4ch).
                    r0 = max(0, g - 4 * ch)
                    w = CH - r0 * P
                    ps_s = spool.tile([P, GROUP * CH], F32, name="ps_s")
                    s_sb = sspool.tile([P, GROUP * CH], BF16, tag="s_sb")
                    ps_v = ps_s.rearrange("p (j w) -> p j w", j=GROUP)
                    s_v = s_sb.rearrange("p (j w) -> p j w", j=GROUP)
                    for j in range(GROUP):
                        kt = g * GROUP + j
                        nc.tensor.matmul(
                            ps_v[:, j, r0 * P:CH],
                            lhsT=kt_sb[:, bass.ts(kt, P)],
                            rhs=qt_sb[:, ch * CH + r0 * P: (ch + 1) * CH],
                            start=True, stop=True)
                    nc.scalar.activation(s_v[:, :, r0 * P:CH],
                                         ps_v[:, :, r0 * P:CH], Exp, scale=0.125)
                    for j in range(GROUP):
                        kt = g * GROUP + j
                        for r in range(r0, 4):
                            d = kt - 8 * ch - 2 * r
                            if 0 <= d <= 1:
                                blk = s_v[:, j, r * P:(r + 1) * P]
                                nc.vector.tensor_tensor(blk, blk, mask_sb[:, d, :], MULT)
                    for j in range(GROUP):
                        kt = g * GROUP + j
                        nc.tensor.matmul(po[0:H + 1, r0 * P:CH],
                                         lhsT=v_sb[:, kt, :],
                                         rhs=s_v[:, j, r0 * P:CH],
                                         start=(kt == 0), stop=(kt == ext - 1))
                    if feeder is not None:
                        for _ in range(per_group):
                            u = next(feeder, None)
                            if u is None:
                                break
                            u()
                if feeder is not None:
                    for u in feeder:
                        u()
                # epilogue
                osb = epool.tile([H + 1, CH], F32, tag="osb")
                nc.vector.tensor_copy(osb[:], po[0:H + 1, :])
                for s in range(CH // P):
                    pt2 = tpool.tile([P, P], F32, tag="tr")
                    nc.tensor.transpose(pt2[:, 0:H + 1], osb[:, bass.ts(s, P)],
                                        ident32[0:H + 1, 0:H + 1])
                    rec = epool.tile([P, 1], F32, tag="rec")
                    nc.vector.reciprocal(rec[:], pt2[:, H:H + 1])
                    ot = epool.tile([P, H], F32, tag="ot")
                    nc.vector.tensor_scalar_mul(ot[:], pt2[:, 0:H], rec[:])
                    nc.sync.dma_start(y[bass.ds(ch * CH + s * P, P), :], ot[:])

    nc.compile()
    return nc


def _shard_inputs(x, Wq, Wk, Wv):
    bf = ml_dtypes.bfloat16
    tri = np.tril(np.ones((P, P), dtype=np.float32)).T  # [kk,qq]=1 iff kk<=qq
    ones = np.ones((P, P), dtype=np.float32)
    zeros = np.zeros((P, P), dtype=np.float32)
    wqq = np.concatenate([Wq.T, Wq.T], axis=1).astype(bf)
    wkv = np.concatenate([Wv.T, Wk.T], axis=1).astype(bf)
    in_maps = []
    for core in range(N_CORES):
        b, p = core // 2, core % 2
        xt = np.ascontiguousarray(x[b].T).astype(bf)
        idx = np.concatenate([np.arange(P * (2 * j + p), P * (2 * j + p) + P)
                              for j in range(16)])
        xtq = np.ascontiguousarray(x[b][idx].T).astype(bf)
        m0 = tri if p == 0 else ones
        m1 = zeros if p == 0 else tri
        m01 = np.concatenate([m0, m1], axis=1).astype(bf)
        in_maps.append({"xt": xt, "xtq": xtq, "wqq": wqq, "wkv": wkv, "m01": m01})
    return in_maps


def _unshard(results):
    y = np.zeros((B, T, H), dtype=np.float32)
    for core in range(N_CORES):
        b, p = core // 2, core % 2
        yc = results[core]["y"]
        for j in range(16):
            g = 2 * j + p
            y[b, P * g:P * g + P] = yc[P * j:P * j + P]
    return y


def kernel(x, Wq, Wk, Wv):
    global LAST_EXEC_TIME_NS, _COMPILED
    x = np.asarray(x, dtype=np.float32)
    Wq = np.asarray(Wq, dtype=np.float32)
    Wk = np.asarray(Wk, dtype=np.float32)
    Wv = np.asarray(Wv, dtype=np.float32)

    if _COMPILED is None:
        _COMPILED = _build_graph()
    nc = _COMPILED

    in_maps = _shard_inputs(x, Wq, Wk, Wv)
    kwargs = {}
    if os.environ.get("ATTN_TRACE"):
        kwargs["trace"] = True
        if os.environ.get("ATTN_TRACE_DIR"):
            kwargs["tmpdir"] = os.environ["ATTN_TRACE_DIR"]
    res = run_bass_kernel_spmd(nc, in_maps, core_ids=list(range(N_CORES)), **kwargs)
    LAST_EXEC_TIME_NS = res.exec_time_ns
    return _unshard(res.results)


# revision 7
# speedup vs baseline: 1.2794x; 1.0356x over previous
"""Single-head causal attention on 8 Trainium2 NeuronCores (Bass/Tile).

Problem: x [4, 4096, 1024] f32, Wq/Wk/Wv [64, 1024] f32 ->
         softmax(causal(q k^T * H^-0.5)) v   -> [4, 4096, 64] f32

Sharding: core = (batch b, parity p), b = core//2, p = core%2. Each core owns
the global 128-wide query tiles g = 2j+p (j=0..15) of its batch -- the parity
interleave balances causal work AND keeps the compiled graph identical across
all 8 cores (SPMD: one NEFF). All parity differences live in host-prepared
data, never in the graph:

  * x arrives as a per-core SHIFTED transpose xt [C, T] whose 128-col key
    blocks are: p=0 -> [zeros | x.T blocks 0..30], p=1 -> [x.T blocks 0..31].
    In this local key space both parities share identical causal geometry:
    local key block k' is fully visible to local query tile r of chunk ch
    (global q-tile g = 8ch+2r+p) iff k' <= 8ch+2r, diagonal (lower-tri mask)
    at k' = 8ch+2r+1, fully masked beyond -- parity-free.
  * The zero-pad block contributes exp(0)*128 = 128 to every softmax
    denominator of p=0 cores; a host-supplied per-core constant (dbias)
    subtracts it exactly before the reciprocal.
  * Queries live in the odd local key blocks (orig g = 2j+p <-> k' = 2j+1),
    so Q projection reads a strided view of the same resident xt -- no
    second copy of x is transferred.

Device pipeline (bf16 matmuls, f32 PSUM accumulation):
  1. xt fully SBUF-resident via 16 large DMAs (2-6 KB lines).
  2. Q^T projection with duplicated weights [Wq.T|Wq.T]; fused [K^T;V^T]
     projection; K^T lands on PSUM rows 64:128 and is repartitioned to rows
     0:64 via SBUF->SBUF DMA (GpSimd ring, off the main DMA stream); V^T ->
     V via PE transposes with a ones-column so the softmax denominator falls
     out of the AV matmul (row 64 of O^T).
  3. Per 512-wide q-chunk ch (extent 8ch+8 k-tiles, in pairs): S^T tiles
     [128k, 512q] on PE -> exp on ScalarE (scale=0.125 folded) -> lower-tri
     mask multiply on the diagonal blocks (DVE) -> AV accumulation. Fully
     masked left col-blocks are suffix-sliced out of S^T/exp/AV. Next
     phase's projection work is drip-fed between groups so ScalarE (the
     critical engine) never starves.
  4. Epilogue per chunk: PE transpose [65,128]->[128,65], subtract dbias,
     reciprocal, scale, DMA out.
"""
import os

import numpy as np
import ml_dtypes

import concourse.bass as bass
import concourse.mybir as mybir
import concourse.tile as tile
from concourse import bacc
from concourse.bass_utils import run_bass_kernel_spmd
from concourse.masks import make_identity

P = 128
B, T, C, H = 4, 4096, 1024, 64
TQ = T // 2          # queries per core
CH = 512             # q-chunk width
NCH = TQ // CH       # 4 q-chunks
CT = C // P          # 8 contraction tiles
TC = T // CH         # 8 t-chunks for K/V proj
NKT = T // P         # 32 k-tiles
GROUP = 2            # k-tiles per exp group
N_CORES = 8

F32 = mybir.dt.float32
BF16 = mybir.dt.bfloat16
Exp = mybir.ActivationFunctionType.Exp
MULT = mybir.AluOpType.mult
SUB = mybir.AluOpType.subtract

LAST_EXEC_TIME_NS = None
_COMPILED = None


def _build_graph():
    nc = bacc.Bacc("TRN2", target_bir_lowering=False, debug=False,
                   num_devices=N_CORES)
    xt = nc.dram_tensor("xt", [C, T], BF16, kind="ExternalInput").ap()
    wqq = nc.dram_tensor("wqq", [C, P], BF16, kind="ExternalInput").ap()
    wkv = nc.dram_tensor("wkv", [C, P], BF16, kind="ExternalInput").ap()
    mtri = nc.dram_tensor("mtri", [P, P], BF16, kind="ExternalInput").ap()
    dbias = nc.dram_tensor("dbias", [P, 1], F32, kind="ExternalInput").ap()
    y = nc.dram_tensor("y", [TQ, H], F32, kind="ExternalOutput").ap()

    xt_r = xt.rearrange("(co p) t -> p co t", p=P)     # [128, 8, 4096]
    wqq_r = wqq.rearrange("(co p) m -> p co m", p=P)   # [128, 8, 128]
    wkv_r = wkv.rearrange("(co p) m -> p co m", p=P)

    with tile.TileContext(nc) as tc:
        with (
            tc.tile_pool(name="const", bufs=1) as const,
            tc.tile_pool(name="ssb", bufs=3) as sspool,
            tc.tile_pool(name="epi", bufs=2) as epool,
            tc.tile_pool(name="pproj", bufs=1, space="PSUM") as ppool,
            tc.tile_pool(name="ps", bufs=2, space="PSUM") as spool,
            tc.tile_pool(name="po", bufs=2, space="PSUM") as opool,
            tc.tile_pool(name="pt", bufs=1, space="PSUM") as tpool,
        ):
            # ---- constants ----
            wqq_sb = const.tile([P, CT, P], BF16, name="wqq_sb")
            wkv_sb = const.tile([P, CT, P], BF16, name="wkv_sb")
            mask_sb = const.tile([P, P], BF16, name="mask_sb")
            dbias_sb = const.tile([P, 1], F32, name="dbias_sb")
            ident16 = const.tile([P, P], BF16, name="ident16")
            ident32 = const.tile([P, P], F32, name="ident32")
            scratch = const.tile([P, 1], F32, name="scratch")
            nc.sync.dma_start(wqq_sb[:], wqq_r)
            nc.sync.dma_start(wkv_sb[:], wkv_r)
            nc.sync.dma_start(mask_sb[:], mtri)
            nc.sync.dma_start(dbias_sb[:], dbias)
            make_identity(nc, ident16[:])
            make_identity(nc, ident32[:])
            # preload the exp table set while projections run
            nc.scalar.activation(scratch[:], ident32[:, 0:1], Exp)

            # ---- resident x ----
            xt_sb = const.tile([P, CT, T], BF16, name="xt_sb")
            # odd local key blocks hold this core's query tokens
            xt_q = xt_sb.rearrange("p co (hb two q) -> p co hb two q",
                                   two=2, q=P)          # [128, 8, 16, 2, 128]

            # ---- persistent activations ----
            qt_sb = const.tile([P, TQ], BF16, name="qt_sb")      # Q^T dup rows
            kt_sb = const.tile([P, T], BF16, name="kt_sb")       # K^T top, zero bottom
            kstage = const.tile([P, T], BF16, name="kstage")     # K^T at rows 64:128
            vt_sb = const.tile([64, T], BF16, name="vt_sb")      # V^T
            v_sb = const.tile([P, NKT, H + 1], BF16, name="v_sb")  # V tiles + ones

            nc.gpsimd.memset(kt_sb[64:128, :], 0.0)
            nc.gpsimd.memset(v_sb[:, :, H:H + 1], 1.0)

            # ---- DMA schedule: big lines (2KB), consumption-ordered waves --
            # wave w covers cols [1024w, 1024w+1024) = the data needed by
            # phase w (Q(w) + KV(2w) + KV(2w+1))
            for w in range(NCH):
                for c in range(CT):
                    nc.sync.dma_start(xt_sb[:, c, bass.ts(w, 2 * CH)],
                                      xt_r[:, c, bass.ts(w, 2 * CH)])

            # ---- projection work units (drip-fed between attention groups) --
            def q_proj_units(qc):
                ps = ppool.tile([P, CH], F32, tag="ps_proj")
                for c in range(CT):
                    yield lambda c=c, ps=ps: nc.tensor.matmul(
                        ps[:], lhsT=wqq_sb[:, c, :],
                        rhs=xt_q[:, c, bass.ts(qc, 4), 1, :],
                        start=(c == 0), stop=(c == CT - 1))
                yield lambda ps=ps: nc.vector.tensor_copy(
                    qt_sb[:, bass.ts(qc, CH)], ps[:])

            def kv_proj_units(t_i):
                ps = ppool.tile([P, CH], F32, tag="ps_proj")
                for c in range(CT):
                    yield lambda c=c, ps=ps: nc.tensor.matmul(
                        ps[:], lhsT=wkv_sb[:, c, :],
                        rhs=xt_sb[:, c, bass.ts(t_i, CH)],
                        start=(c == 0), stop=(c == CT - 1))

                def evac(ps=ps):
                    nc.vector.tensor_copy(vt_sb[:, bass.ts(t_i, CH)], ps[0:64, :])
                    nc.vector.tensor_copy(kstage[64:128, bass.ts(t_i, CH)],
                                          ps[64:128, :])
                    nc.gpsimd.dma_start(kt_sb[0:64, bass.ts(t_i, CH)],
                                        kstage[64:128, bass.ts(t_i, CH)])
                yield evac
                for j in range(CH // P):
                    def vtile(j=j):
                        kt = t_i * (CH // P) + j
                        pt = tpool.tile([P, P], BF16, tag="tr")
                        nc.tensor.transpose(pt[:, 0:64], vt_sb[:, bass.ts(kt, P)],
                                            ident16[0:64, 0:64])
                        nc.vector.tensor_copy(v_sb[:, kt, 0:H], pt[:, 0:64])
                    yield vtile

            def phase_units(phase):
                yield from q_proj_units(phase)
                yield from kv_proj_units(2 * phase)
                yield from kv_proj_units(2 * phase + 1)

            # ---- attention ----
            for u in phase_units(0):
                u()
            for ch in range(NCH):
                ext = 8 * ch + 8
                n_groups = ext // GROUP
                if ch + 1 < NCH:
                    feeder = phase_units(ch + 1)
                    per_group = 35 // n_groups + 1
                else:
                    feeder = None
                po = opool.tile([P, CH], F32, name="po")
                for g in range(n_groups):
                    # left col-blocks with k'-8ch-2r >= 2 are fully masked:
                    # suffix-slice them out of S^T, exp and AV. Both k-tiles
                    # of the pair share r0 = max(0, g - 4ch).
                    r0 = max(0, g - 4 * ch)
                    ps_s = spool.tile([P, GROUP * CH], F32, name="ps_s")
                    s_sb = sspool.tile([P, GROUP * CH], BF16, tag="s_sb")
                    ps_v = ps_s.rearrange("p (j w) -> p j w", j=GROUP)
                    s_v = s_sb.rearrange("p (j w) -> p j w", j=GROUP)
                    for j in range(GROUP):
                        kt = g * GROUP + j
                        nc.tensor.matmul(
                            ps_v[:, j, r0 * P:CH],
                            lhsT=kt_sb[:, bass.ts(kt, P)],
                            rhs=qt_sb[:, ch * CH + r0 * P: (ch + 1) * CH],
                            start=True, stop=True)
                    nc.scalar.activation(s_v[:, :, r0 * P:CH],
                                         ps_v[:, :, r0 * P:CH], Exp, scale=0.125)
                    for j in range(GROUP):
                        kt = g * GROUP + j
                        for r in range(r0, 4):
                            if kt - 8 * ch - 2 * r == 1:  # diagonal block
                                blk = s_v[:, j, r * P:(r + 1) * P]
                                nc.vector.tensor_tensor(blk, blk, mask_sb[:], MULT)
                    for j in range(GROUP):
                        kt = g * GROUP + j
                        nc.tensor.matmul(po[0:H + 1, r0 * P:CH],
                                         lhsT=v_sb[:, kt, :],
                                         rhs=s_v[:, j, r0 * P:CH],
                                         start=(kt == 0), stop=(kt == ext - 1))
                    if feeder is not None:
                        for _ in range(per_group):
                            u = next(feeder, None)
                            if u is None:
                                break
                            u()
                if feeder is not None:
                    for u in feeder:
                        u()
                # epilogue
                osb = epool.tile([H + 1, CH], F32, tag="osb")
                nc.vector.tensor_copy(osb[:], po[0:H + 1, :])
                for s in range(CH // P):
                    pt2 = tpool.tile([P, P], F32, tag="tr")
                    nc.tensor.transpose(pt2[:, 0:H + 1], osb[:, bass.ts(s, P)],
                                        ident32[0:H + 1, 0:H + 1])
                    den = epool.tile([P, 1], F32, tag="den")
                    nc.vector.tensor_tensor(den[:], pt2[:, H:H + 1], dbias_sb[:], SUB)
                    rec = epool.tile([P, 1], F32, tag="rec")
                    nc.vector.reciprocal(rec[:], den[:])
                    ot = epool.tile([P, H], F32, tag="ot")
                    nc.vector.tensor_scalar_mul(ot[:], pt2[:, 0:H], rec[:])
                    nc.gpsimd.dma_start(y[bass.ds(ch * CH + s * P, P), :], ot[:])

    nc.compile()
    return nc


def _shard_inputs(x, Wq, Wk, Wv):
    bf = ml_dtypes.bfloat16
    tri = np.tril(np.ones((P, P), dtype=np.float32)).T  # [kk,qq]=1 iff kk<=qq
    wqq = np.concatenate([Wq.T, Wq.T], axis=1).astype(bf)
    wkv = np.concatenate([Wv.T, Wk.T], axis=1).astype(bf)
    mtri = tri.astype(bf)
    in_maps = []
    for core in range(N_CORES):
        b, p = core // 2, core % 2
        if p == 0:
            # [zeros | blocks 0..30]
            xt_full = np.concatenate(
                [np.zeros((P, C), dtype=np.float32), x[b][:T - P]], axis=0).T
        else:
            xt_full = x[b].T
        xt_core = np.ascontiguousarray(xt_full).astype(bf)
        db = np.full((P, 1), 128.0 if p == 0 else 0.0, dtype=np.float32)
        in_maps.append({"xt": xt_core, "wqq": wqq, "wkv": wkv,
                        "mtri": mtri, "dbias": db})
    return in_maps


def _unshard(results):
    y = np.zeros((B, T, H), dtype=np.float32)
    for core in range(N_CORES):
        b, p = core // 2, core % 2
        yc = results[core]["y"]
        for j in range(16):
            g = 2 * j + p
            y[b, P * g:P * g + P] = yc[P * j:P * j + P]
    return y


def kernel(x, Wq, Wk, Wv):
    global LAST_EXEC_TIME_NS, _COMPILED
    x = np.asarray(x, dtype=np.float32)
    Wq = np.asarray(Wq, dtype=np.float32)
    Wk = np.asarray(Wk, dtype=np.float32)
    Wv = np.asarray(Wv, dtype=np.float32)

    if _COMPILED is None:
        _COMPILED = _build_graph()
    nc = _COMPILED

    in_maps = _shard_inputs(x, Wq, Wk, Wv)
    kwargs = {}
    if os.environ.get("ATTN_TRACE"):
        kwargs["trace"] = True
        if os.environ.get("ATTN_TRACE_DIR"):
            kwargs["tmpdir"] = os.environ["ATTN_TRACE_DIR"]
    res = run_bass_kernel_spmd(nc, in_maps, core_ids=list(range(N_CORES)), **kwargs)
    LAST_EXEC_TIME_NS = res.exec_time_ns
    return _unshard(res.results)


# revision 8
# speedup vs baseline: 1.3518x; 1.0566x over previous
"""Single-head causal attention on 8 Trainium2 NeuronCores (Bass/Tile).

Problem: x [4, 4096, 1024] f32, Wq/Wk/Wv [64, 1024] f32 ->
         softmax(causal(q k^T * H^-0.5)) v   -> [4, 4096, 64] f32

Sharding: core = (batch b, parity p), b = core//2, p = core%2. Each core owns
the global 128-wide query tiles g = 2j+p (j=0..15) of its batch -- the parity
interleave balances causal work AND keeps the compiled graph identical across
all 8 cores (SPMD: one NEFF). All parity differences live in host-prepared
data, never in the graph:

  * x arrives as a per-core SHIFTED transpose xt [C, T] whose 128-col key
    blocks are: p=0 -> [zeros | x.T blocks 0..30], p=1 -> [x.T blocks 0..31].
    In this local key space both parities share identical causal geometry:
    local key block k' is fully visible to local query tile r of chunk ch
    (global q-tile g = 8ch+2r+p) iff k' <= 8ch+2r, diagonal (lower-tri mask)
    at k' = 8ch+2r+1, fully masked beyond -- parity-free.
  * The zero-pad block contributes exp(0)*128 = 128 to every softmax
    denominator of p=0 cores; a host-supplied per-core constant (dbias)
    subtracts it exactly before the reciprocal.
  * Queries live in the odd local key blocks (orig g = 2j+p <-> k' = 2j+1),
    so Q projection reads a strided view of the same resident xt -- no
    second copy of x is transferred.

Device pipeline (bf16 matmuls, f32 PSUM accumulation):
  1. xt fully SBUF-resident via 16 large DMAs (2-6 KB lines).
  2. Q^T projection with duplicated weights [Wq.T|Wq.T]; fused [K^T;V^T]
     projection; K^T lands on PSUM rows 64:128 and is repartitioned to rows
     0:64 via SBUF->SBUF DMA (GpSimd ring, off the main DMA stream); V^T ->
     V via PE transposes with a ones-column so the softmax denominator falls
     out of the AV matmul (row 64 of O^T).
  3. Per 512-wide q-chunk ch (extent 8ch+8 k-tiles, in pairs): S^T tiles
     [128k, 512q] on PE -> exp on ScalarE (scale=0.125 folded) -> lower-tri
     mask multiply on the diagonal blocks (DVE) -> AV accumulation. Fully
     masked left col-blocks are suffix-sliced out of S^T/exp/AV. Next
     phase's projection work is drip-fed between groups so ScalarE (the
     critical engine) never starves.
  4. Epilogue per chunk: PE transpose [65,128]->[128,65], subtract dbias,
     reciprocal, scale, DMA out.
"""
import os

import numpy as np
import ml_dtypes

import concourse.bass as bass
import concourse.mybir as mybir
import concourse.tile as tile
from concourse import bacc
from concourse.bass_utils import run_bass_kernel_spmd
from concourse.masks import make_identity

P = 128
B, T, C, H = 4, 4096, 1024, 64
TQ = T // 2          # queries per core
CH = 512             # q-chunk width
NCH = TQ // CH       # 4 q-chunks
CT = C // P          # 8 contraction tiles
TC = T // CH         # 8 t-chunks for K/V proj
NKT = T // P         # 32 k-tiles
GROUP = 2            # k-tiles per exp group
N_CORES = 8

F32 = mybir.dt.float32
BF16 = mybir.dt.bfloat16
Exp = mybir.ActivationFunctionType.Exp
MULT = mybir.AluOpType.mult
SUB = mybir.AluOpType.subtract

LAST_EXEC_TIME_NS = None
_COMPILED = None


def _build_graph():
    nc = bacc.Bacc("TRN2", target_bir_lowering=False, debug=False,
                   num_devices=N_CORES)
    xt = nc.dram_tensor("xt", [C, T], BF16, kind="ExternalInput").ap()
    wqq = nc.dram_tensor("wqq", [C, P], BF16, kind="ExternalInput").ap()
    wkv = nc.dram_tensor("wkv", [C, P], BF16, kind="ExternalInput").ap()
    mtri = nc.dram_tensor("mtri", [P, P], BF16, kind="ExternalInput").ap()
    dbias = nc.dram_tensor("dbias", [P, 1], F32, kind="ExternalInput").ap()
    y = nc.dram_tensor("y", [TQ, H], F32, kind="ExternalOutput").ap()

    xt_r = xt.rearrange("(co p) t -> p co t", p=P)     # [128, 8, 4096]
    wqq_r = wqq.rearrange("(co p) m -> p co m", p=P)   # [128, 8, 128]
    wkv_r = wkv.rearrange("(co p) m -> p co m", p=P)

    with tile.TileContext(nc) as tc:
        with (
            tc.tile_pool(name="const", bufs=1) as const,
            tc.tile_pool(name="ssb", bufs=3) as sspool,
            tc.tile_pool(name="epi", bufs=2) as epool,
            tc.tile_pool(name="pproj", bufs=2, space="PSUM") as ppool,
            tc.tile_pool(name="ps", bufs=2, space="PSUM") as spool,
            tc.tile_pool(name="po", bufs=1, space="PSUM") as opool,
            tc.tile_pool(name="pt", bufs=1, space="PSUM") as tpool,
        ):
            # ---- constants ----
            wqq_sb = const.tile([P, CT, P], BF16, name="wqq_sb")
            wkv_sb = const.tile([P, CT, P], BF16, name="wkv_sb")
            mask_sb = const.tile([P, P], BF16, name="mask_sb")
            dbias_sb = const.tile([P, 1], F32, name="dbias_sb")
            ident16 = const.tile([P, P], BF16, name="ident16")
            ident32 = const.tile([P, P], F32, name="ident32")
            scratch = const.tile([P, 1], F32, name="scratch")
            nc.sync.dma_start(wqq_sb[:], wqq_r)
            nc.sync.dma_start(wkv_sb[:], wkv_r)
            nc.sync.dma_start(mask_sb[:], mtri)
            nc.sync.dma_start(dbias_sb[:], dbias)
            make_identity(nc, ident16[:])
            make_identity(nc, ident32[:])
            # preload the exp table set while projections run
            nc.scalar.activation(scratch[:], ident32[:, 0:1], Exp)

            # ---- resident x ----
            xt_sb = const.tile([P, CT, T], BF16, name="xt_sb")
            # odd local key blocks hold this core's query tokens
            xt_q = xt_sb.rearrange("p co (hb two q) -> p co hb two q",
                                   two=2, q=P)          # [128, 8, 16, 2, 128]

            # ---- persistent activations ----
            qt_sb = const.tile([P, TQ], BF16, name="qt_sb")      # Q^T dup rows
            kt_sb = const.tile([P, T], BF16, name="kt_sb")       # K^T top, zero bottom
            kstage = const.tile([P, T], BF16, name="kstage")     # K^T at rows 64:128
            vt_sb = const.tile([64, T], BF16, name="vt_sb")      # V^T
            v_sb = const.tile([P, NKT, H + 1], BF16, name="v_sb")  # V tiles + ones

            nc.gpsimd.memset(kt_sb[64:128, :], 0.0)
            nc.gpsimd.memset(v_sb[:, :, H:H + 1], 1.0)

            # ---- DMA schedule: big lines (2KB), consumption-ordered waves --
            # wave w covers cols [1024w, 1024w+1024) = the data needed by
            # phase w (Q(w) + KV(2w) + KV(2w+1))
            for w in range(NCH):
                for c in range(CT):
                    for h in range(2):
                        nc.sync.dma_start(
                            xt_sb[64 * h:64 * (h + 1), c, bass.ts(w, 2 * CH)],
                            xt_r[64 * h:64 * (h + 1), c, bass.ts(w, 2 * CH)])

            # ---- projection work units (drip-fed between attention groups) --
            def q_proj_units(qc):
                ps = ppool.tile([P, CH], F32, tag="ps_proj")
                for c in range(CT):
                    yield lambda c=c, ps=ps: nc.tensor.matmul(
                        ps[:], lhsT=wqq_sb[:, c, :],
                        rhs=xt_q[:, c, bass.ts(qc, 4), 1, :],
                        start=(c == 0), stop=(c == CT - 1))
                yield lambda ps=ps: nc.vector.tensor_copy(
                    qt_sb[:, bass.ts(qc, CH)], ps[:])

            def kv_proj_units(t_i):
                ps = ppool.tile([P, CH], F32, tag="ps_proj")
                for c in range(CT):
                    yield lambda c=c, ps=ps: nc.tensor.matmul(
                        ps[:], lhsT=wkv_sb[:, c, :],
                        rhs=xt_sb[:, c, bass.ts(t_i, CH)],
                        start=(c == 0), stop=(c == CT - 1))

                def evac_k(ps=ps):
                    nc.vector.tensor_copy(kstage[64:128, bass.ts(t_i, CH)],
                                          ps[64:128, :])
                    nc.gpsimd.dma_start(kt_sb[0:64, bass.ts(t_i, CH)],
                                        kstage[64:128, bass.ts(t_i, CH)])
                yield evac_k
                yield lambda ps=ps: nc.vector.tensor_copy(
                    vt_sb[:, bass.ts(t_i, CH)], ps[0:64, :])
                for j in range(CH // P):
                    def vtile(j=j):
                        kt = t_i * (CH // P) + j
                        pt = tpool.tile([P, P], BF16, tag="tr")
                        nc.tensor.transpose(pt[:, 0:64], vt_sb[:, bass.ts(kt, P)],
                                            ident16[0:64, 0:64])
                        nc.vector.tensor_copy(v_sb[:, kt, 0:H], pt[:, 0:64])
                    yield vtile

            def phase_units(phase):
                yield from q_proj_units(phase)
                yield from kv_proj_units(2 * phase)
                yield from kv_proj_units(2 * phase + 1)

            # ---- attention ----
            for u in phase_units(0):
                u()
            for ch in range(NCH):
                ext = 8 * ch + 8
                n_groups = ext // GROUP
                if ch + 1 < NCH:
                    feeder = phase_units(ch + 1)
                    per_group = 35 // n_groups + 1
                else:
                    feeder = None
                po = opool.tile([P, CH], F32, name="po")
                for g in range(n_groups):
                    # left col-blocks with k'-8ch-2r >= 2 are fully masked:
                    # suffix-slice them out of S^T, exp and AV. Both k-tiles
                    # of the pair share r0 = max(0, g - 4ch).
                    r0 = max(0, g - 4 * ch)
                    ps_s = spool.tile([P, GROUP * CH], F32, name="ps_s")
                    s_sb = sspool.tile([P, GROUP * CH], BF16, tag="s_sb")
                    ps_v = ps_s.rearrange("p (j w) -> p j w", j=GROUP)
                    s_v = s_sb.rearrange("p (j w) -> p j w", j=GROUP)
                    for j in range(GROUP):
                        kt = g * GROUP + j
                        nc.tensor.matmul(
                            ps_v[:, j, r0 * P:CH],
                            lhsT=kt_sb[:, bass.ts(kt, P)],
                            rhs=qt_sb[:, ch * CH + r0 * P: (ch + 1) * CH],
                            start=True, stop=True)
                    nc.scalar.activation(s_v[:, :, r0 * P:CH],
                                         ps_v[:, :, r0 * P:CH], Exp, scale=0.125)
                    for j in range(GROUP):
                        kt = g * GROUP + j
                        for r in range(r0, 4):
                            if kt - 8 * ch - 2 * r == 1:  # diagonal block
                                blk = s_v[:, j, r * P:(r + 1) * P]
                                nc.vector.tensor_tensor(blk, blk, mask_sb[:], MULT)
                    for j in range(GROUP):
                        kt = g * GROUP + j
                        nc.tensor.matmul(po[0:H + 1, r0 * P:CH],
                                         lhsT=v_sb[:, kt, :],
                                         rhs=s_v[:, j, r0 * P:CH],
                                         start=(kt == 0), stop=(kt == ext - 1))
                    if feeder is not None:
                        for _ in range(per_group):
                            u = next(feeder, None)
                            if u is None:
                                break
                            u()
                if feeder is not None:
                    for u in feeder:
                        u()
                # epilogue
                osb = epool.tile([H + 1, CH], F32, tag="osb")
                nc.vector.tensor_copy(osb[:], po[0:H + 1, :])
                for s in range(CH // P):
                    pt2 = tpool.tile([P, P], F32, tag="tr")
                    nc.tensor.transpose(pt2[:, 0:H + 1], osb[:, bass.ts(s, P)],
                                        ident32[0:H + 1, 0:H + 1])
                    den = epool.tile([P, 1], F32, tag="den")
                    nc.vector.tensor_tensor(den[:], pt2[:, H:H + 1], dbias_sb[:], SUB)
                    rec = epool.tile([P, 1], F32, tag="rec")
                    nc.vector.reciprocal(rec[:], den[:])
                    ot = epool.tile([P, H], F32, tag="ot")
                    nc.vector.tensor_scalar_mul(ot[:], pt2[:, 0:H], rec[:])
                    nc.gpsimd.dma_start(y[bass.ds(ch * CH + s * P, P), :], ot[:])

    nc.compile()
    return nc


def _shard_inputs(x, Wq, Wk, Wv):
    bf = ml_dtypes.bfloat16
    tri = np.tril(np.ones((P, P), dtype=np.float32)).T  # [kk,qq]=1 iff kk<=qq
    wqq = np.concatenate([Wq.T, Wq.T], axis=1).astype(bf)
    wkv = np.concatenate([Wv.T, Wk.T], axis=1).astype(bf)
    mtri = tri.astype(bf)
    in_maps = []
    for core in range(N_CORES):
        b, p = core // 2, core % 2
        if p == 0:
            # [zeros | blocks 0..30]
            xt_full = np.concatenate(
                [np.zeros((P, C), dtype=np.float32), x[b][:T - P]], axis=0).T
        else:
            xt_full = x[b].T
        xt_core = np.ascontiguousarray(xt_full).astype(bf)
        db = np.full((P, 1), 128.0 if p == 0 else 0.0, dtype=np.float32)
        in_maps.append({"xt": xt_core, "wqq": wqq, "wkv": wkv,
                        "mtri": mtri, "dbias": db})
    return in_maps


def _unshard(results):
    y = np.zeros((B, T, H), dtype=np.float32)
    for core in range(N_CORES):
        b, p = core // 2, core % 2
        yc = results[core]["y"]
        for j in range(16):
            g = 2 * j + p
            y[b, P * g:P * g + P] = yc[P * j:P * j + P]
    return y


def kernel(x, Wq, Wk, Wv):
    global LAST_EXEC_TIME_NS, _COMPILED
    x = np.asarray(x, dtype=np.float32)
    Wq = np.asarray(Wq, dtype=np.float32)
    Wk = np.asarray(Wk, dtype=np.float32)
    Wv = np.asarray(Wv, dtype=np.float32)

    if _COMPILED is None:
        _COMPILED = _build_graph()
    nc = _COMPILED

    in_maps = _shard_inputs(x, Wq, Wk, Wv)
    kwargs = {}
    if os.environ.get("ATTN_TRACE"):
        kwargs["trace"] = True
        if os.environ.get("ATTN_TRACE_DIR"):
            kwargs["tmpdir"] = os.environ["ATTN_TRACE_DIR"]
    res = run_bass_kernel_spmd(nc, in_maps, core_ids=list(range(N_CORES)), **kwargs)
    LAST_EXEC_TIME_NS = res.exec_time_ns
    return _unshard(res.results)
